# revision 18
# baseline (speedup 1.0000x reference)
"""HardAttention kernel for Trainium2 (8 NeuronCores, Bass/Tile).

reference:
    scores = einsum("btd,bcsd->btcs", xs, ys)   # (B,Tx,C,Ty)
    out    = scores.max(-1).sum(1)              # (B,C)

Shapes: B=16, Tx=128, C=64, Ty=128, d=768.

Strategy:
  - Data-parallel over B: core i handles batches [2i, 2i+2).
  - Host pre-casts both operands to fp8 e4m3 (measured end-to-end rel
    err ~4e-3 vs the 2e-2 gate) and lays them out dk-major (partition
    dim first, contiguous DMA lines):
        xsT[dk, b, k, t]    = xs[b, t, 128k+dk]     (128, B, KC, Tx)
        ysT[b, dk, k, c, s] = ys[b, c, s, 128k+dk]  (B, 128, KC, C, Ty)
    so every DMA is a plain HWDGE copy (no dtype cast in the DMA path —
    SWDGE casting DMAs emit per-element descriptors and run ~5 GB/s).
  - Per (b, quarter-of-16-candidates): one 1.5 MB HWDGE DMA (contiguous
    2 KB runs per partition), then 3 (k-pair) x 4 (groups of 4
    candidates -> N=512) DoubleRow fp8 matmuls (2 contraction chunks
    per instruction, 0.5 cyc/row) accumulating into 4 PSUM banks; DVE
    reduce_max over Ty per candidate into an SBUF tile M[t, c]; finally
    a ones-vector matmul contracts the partition axis (sum over t) ->
    out[b, c].
"""

import os

import numpy as np

B, TX, C, TY, D = 16, 128, 64, 128, 768
N_CORES = 8
BPC = B // N_CORES          # batches per core = 2
KC = D // 128               # contraction chunks = 6
QC = 32                     # candidates per slab (DMA granule)
NQ = C // QC                # slabs per batch = 2
G = 4                       # candidates per matmul (N = G*TY = 512)

_CACHE = {}
LAST_RESULTS = None


def _build():
    import concourse.bass as bass
    import concourse.mybir as mybir
    import concourse.tile as tile
    from concourse import bacc

    fp8 = mybir.dt.float8e4
    f32 = mybir.dt.float32

    nc = bacc.Bacc(
        "TRN2",
        target_bir_lowering=False,
        debug=False,
        num_devices=N_CORES,
    )

    xs_ap = nc.dram_tensor("xsT", (128, BPC, KC, TX), fp8, kind="ExternalInput").ap()
    ys_ap = nc.dram_tensor(
        "ysT", (BPC, NQ, 128, KC, QC, TY), fp8, kind="ExternalInput"
    ).ap()
    out_ap = nc.dram_tensor("out", (1, BPC * C), f32, kind="ExternalOutput").ap()

    with tile.TileContext(nc) as tc:
        with (
            tc.tile_pool(name="xt", bufs=1) as xpool,
            tc.tile_pool(name="yt", bufs=2 * NQ) as ypool,  # 4 x 24KB/part
            tc.tile_pool(name="mt", bufs=1) as mpool,
            tc.tile_pool(name="ones", bufs=1) as opool,
            tc.tile_pool(name="osb", bufs=1) as obpool,
            tc.tile_pool(name="ps", bufs=6, space="PSUM") as pspool,
            tc.tile_pool(name="pso", bufs=1, space="PSUM") as psopool,
        ):
            # All of xsT for this core: contiguous 1.5 KB per partition.
            xt = xpool.tile([128, BPC, KC, TX], fp8)
            nc.scalar.dma_start(xt[:], xs_ap)

            ones = opool.tile([128, 1], f32)
            nc.any.memset(ones[:], 1.0)

            m_all = mpool.tile([128, BPC, C], f32)  # max_s scores, [t, b, c]
            for b in range(BPC):
                for q in range(NQ):
                    # slab: 12 KB fully contiguous per partition
                    yt = ypool.tile([128, KC, QC, TY], fp8)
                    dma_eng = nc.sync if (b * NQ + q) % 2 == 0 else nc.scalar
                    dma_eng.dma_start(yt[:], ys_ap[b, q])
                    psums = [
                        pspool.tile([128, G, TY], f32, name=f"ps_{b}_{q}_{g}", tag="ps")
                        for g in range(QC // G)
                    ]
                    # group-major order: each group finishes (and frees its
                    # PSUM bank via its reduce) before later groups need one
                    for g in range(QC // G):
                        for j in range(KC // 2):
                            nc.tensor.matmul(
                                psums[g][:],
                                lhsT=xt[:, b, 2 * j : 2 * j + 2, :],
                                rhs=yt[:, 2 * j : 2 * j + 2, g * G : (g + 1) * G, :],
                                start=(j == 0),
                                stop=(j == KC // 2 - 1),
                                perf_mode=mybir.MatmulPerfMode.DoubleRow,
                            )
                        nc.vector.reduce_max(
                            m_all[:, b, q * QC + g * G : q * QC + (g + 1) * G],
                            psums[g][:],
                            axis=mybir.AxisListType.X,
                        )
            # sum over t (partition axis) via ones-vector matmul, both b at once
            out_ps = psopool.tile([1, BPC * C], f32, tag="out_ps")
            nc.tensor.matmul(
                out_ps[:], lhsT=ones[:], rhs=m_all[:], start=True, stop=True
            )
            osb = obpool.tile([1, BPC * C], f32, tag="osb")
            nc.vector.tensor_copy(osb[:], out_ps[:])
            nc.sync.dma_start(out_ap, osb[:])

    nc.compile()
    return nc


def _get_nc():
    if "nc" not in _CACHE:
        _CACHE["nc"] = _build()
    return _CACHE["nc"]


def _prep(xs: np.ndarray, ys: np.ndarray):
    """Host-side layout: fp8 cast + dk-major transpose (XLA on CPU)."""
    import jax
    import jax.numpy as jnp
    import ml_dtypes

    fp8 = ml_dtypes.float8_e4m3

    def _f(xs, ys):
        # xsT[dk, b, k, t] = xs[b, t, 128k+dk]
        xsT = jnp.transpose(
            jnp.reshape(xs.astype(fp8), (B, TX, KC, 128)), (3, 0, 2, 1)
        )
        # ysT[b, q, dk, k, cq, s] = ys[b, q*QC+cq, s, 128k+dk]
        ysT = jnp.transpose(
            jnp.reshape(ys.astype(fp8), (B, NQ, QC, TY, KC, 128)), (0, 1, 5, 4, 2, 3)
        )
        return xsT, ysT

    cpu = jax.devices("cpu")[0]
    with jax.default_device(cpu):
        xs_c = jax.device_put(np.ascontiguousarray(xs, dtype=np.float32), cpu)
        ys_c = jax.device_put(np.ascontiguousarray(ys, dtype=np.float32), cpu)
        xsT, ysT = jax.jit(_f)(xs_c, ys_c)
        return np.asarray(xsT), np.asarray(ysT)


def kernel(xs: np.ndarray, ys: np.ndarray) -> np.ndarray:
    global LAST_RESULTS
    from concourse.bass_utils import run_bass_kernel_spmd

    nc = _get_nc()
    xsT, ysT = _prep(xs, ys)
    in_maps = [
        {
            "xsT": np.ascontiguousarray(xsT[:, i * BPC : (i + 1) * BPC]),
            "ysT": ysT[i * BPC : (i + 1) * BPC],
        }
        for i in range(N_CORES)
    ]
    res = run_bass_kernel_spmd(
        nc,
        in_maps,
        core_ids=list(range(N_CORES)),
        tmpdir=os.environ.get("KERNEL_TMPDIR"),
    )
    LAST_RESULTS = res
    out = np.concatenate(
        [res.results[i]["out"].reshape(BPC, C) for i in range(N_CORES)], axis=0
    )
    return out.astype(np.float32)


# revision 19
# speedup vs baseline: 1.0499x; 1.0499x over previous
"""HardAttention kernel for Trainium2 (8 NeuronCores, Bass/Tile).

reference:
    scores = einsum("btd,bcsd->btcs", xs, ys)   # (B,Tx,C,Ty)
    out    = scores.max(-1).sum(1)              # (B,C)

Shapes: B=16, Tx=128, C=64, Ty=128, d=768.

Strategy:
  - Data-parallel over B: core i handles batches [2i, 2i+2).
  - Host pre-casts both operands to fp8 e4m3 (measured end-to-end rel
    err ~4e-3 vs the 2e-2 gate) and lays them out dk-major (partition
    dim first, contiguous DMA lines):
        xsT[dk, b, k, t]    = xs[b, t, 128k+dk]     (128, B, KC, Tx)
        ysT[b, dk, k, c, s] = ys[b, c, s, 128k+dk]  (B, 128, KC, C, Ty)
    so every DMA is a plain HWDGE copy (no dtype cast in the DMA path —
    SWDGE casting DMAs emit per-element descriptors and run ~5 GB/s).
  - Per (b, quarter-of-16-candidates): one 1.5 MB HWDGE DMA (contiguous
    2 KB runs per partition), then 3 (k-pair) x 4 (groups of 4
    candidates -> N=512) DoubleRow fp8 matmuls (2 contraction chunks
    per instruction, 0.5 cyc/row) accumulating into 4 PSUM banks; DVE
    reduce_max over Ty per candidate into an SBUF tile M[t, c]; finally
    a ones-vector matmul contracts the partition axis (sum over t) ->
    out[b, c].
"""

import os

import numpy as np

B, TX, C, TY, D = 16, 128, 64, 128, 768
N_CORES = 8
BPC = B // N_CORES          # batches per core = 2
KC = D // 128               # contraction chunks = 6
QC = 32                     # candidates per slab (DMA granule)
NQ = C // QC                # slabs per batch = 2
G = 4                       # candidates per matmul (N = G*TY = 512)

_CACHE = {}
LAST_RESULTS = None


def _build():
    import concourse.bass as bass
    import concourse.mybir as mybir
    import concourse.tile as tile
    from concourse import bacc

    fp8 = mybir.dt.float8e4
    f32 = mybir.dt.float32

    nc = bacc.Bacc(
        "TRN2",
        target_bir_lowering=False,
        debug=False,
        num_devices=N_CORES,
    )

    xs_ap = nc.dram_tensor("xsT", (128, BPC, KC, TX), fp8, kind="ExternalInput").ap()
    ys_ap = nc.dram_tensor(
        "ysT", (BPC, NQ, 128, KC, QC, TY), fp8, kind="ExternalInput"
    ).ap()
    out_ap = nc.dram_tensor("out", (1, BPC * C), f32, kind="ExternalOutput").ap()

    with tile.TileContext(nc) as tc:
        with (
            tc.tile_pool(name="xt", bufs=1) as xpool,
            tc.tile_pool(name="yt", bufs=2 * NQ) as ypool,  # 4 x 24KB/part
            tc.tile_pool(name="mt", bufs=1) as mpool,
            tc.tile_pool(name="ones", bufs=1) as opool,
            tc.tile_pool(name="osb", bufs=1) as obpool,
            tc.tile_pool(name="ps", bufs=6, space="PSUM") as pspool,
            tc.tile_pool(name="pso", bufs=1, space="PSUM") as psopool,
        ):
            # All of xsT for this core: contiguous 1.5 KB per partition.
            xt = xpool.tile([128, BPC, KC, TX], fp8)
            nc.scalar.dma_start(xt[:], xs_ap)

            ones = opool.tile([128, 1], f32)
            nc.any.memset(ones[:], 1.0)

            m_all = mpool.tile([128, BPC, C], f32)  # max_s scores, [t, b, c]
            for b in range(BPC):
                for q in range(NQ):
                    # slab: 12 KB fully contiguous per partition
                    yt = ypool.tile([128, KC, QC, TY], fp8)
                    dma_eng = nc.sync if (b * NQ + q) % 2 == 0 else nc.scalar
                    dma_eng.dma_start(yt[:], ys_ap[b, q])
                    # two half-slab passes, j-major within each so the 4
                    # PSUM banks give cross-bank matmul ILP
                    for h in range(QC // (4 * G)):
                        psums = [
                            pspool.tile(
                                [128, G, TY], f32, name=f"ps_{b}_{q}_{h}_{g}", tag="ps"
                            )
                            for g in range(4)
                        ]
                        for j in range(KC // 2):
                            for g in range(4):
                                gg = 4 * h + g
                                nc.tensor.matmul(
                                    psums[g][:],
                                    lhsT=xt[:, b, 2 * j : 2 * j + 2, :],
                                    rhs=yt[:, 2 * j : 2 * j + 2, gg * G : (gg + 1) * G, :],
                                    start=(j == 0),
                                    stop=(j == KC // 2 - 1),
                                    perf_mode=mybir.MatmulPerfMode.DoubleRow,
                                )
                        for g in range(4):
                            gg = 4 * h + g
                            nc.vector.reduce_max(
                                m_all[:, b, q * QC + gg * G : q * QC + (gg + 1) * G],
                                psums[g][:],
                                axis=mybir.AxisListType.X,
                            )
            # sum over t (partition axis) via ones-vector matmul, both b at once
            out_ps = psopool.tile([1, BPC * C], f32, tag="out_ps")
            nc.tensor.matmul(
                out_ps[:], lhsT=ones[:], rhs=m_all[:], start=True, stop=True
            )
            osb = obpool.tile([1, BPC * C], f32, tag="osb")
            nc.vector.tensor_copy(osb[:], out_ps[:])
            nc.sync.dma_start(out_ap, osb[:])

    nc.compile()
    return nc


def _get_nc():
    if "nc" not in _CACHE:
        _CACHE["nc"] = _build()
    return _CACHE["nc"]


def _prep(xs: np.ndarray, ys: np.ndarray):
    """Host-side layout: fp8 cast + dk-major transpose (XLA on CPU)."""
    import jax
    import jax.numpy as jnp
    import ml_dtypes

    fp8 = ml_dtypes.float8_e4m3

    def _f(xs, ys):
        # xsT[dk, b, k, t] = xs[b, t, 128k+dk]
        xsT = jnp.transpose(
            jnp.reshape(xs.astype(fp8), (B, TX, KC, 128)), (3, 0, 2, 1)
        )
        # ysT[b, q, dk, k, cq, s] = ys[b, q*QC+cq, s, 128k+dk]
        ysT = jnp.transpose(
            jnp.reshape(ys.astype(fp8), (B, NQ, QC, TY, KC, 128)), (0, 1, 5, 4, 2, 3)
        )
        return xsT, ysT

    cpu = jax.devices("cpu")[0]
    with jax.default_device(cpu):
        xs_c = jax.device_put(np.ascontiguousarray(xs, dtype=np.float32), cpu)
        ys_c = jax.device_put(np.ascontiguousarray(ys, dtype=np.float32), cpu)
        xsT, ysT = jax.jit(_f)(xs_c, ys_c)
        return np.asarray(xsT), np.asarray(ysT)


def kernel(xs: np.ndarray, ys: np.ndarray) -> np.ndarray:
    global LAST_RESULTS
    from concourse.bass_utils import run_bass_kernel_spmd

    nc = _get_nc()
    xsT, ysT = _prep(xs, ys)
    in_maps = [
        {
            "xsT": np.ascontiguousarray(xsT[:, i * BPC : (i + 1) * BPC]),
            "ysT": ysT[i * BPC : (i + 1) * BPC],
        }
        for i in range(N_CORES)
    ]
    res = run_bass_kernel_spmd(
        nc,
        in_maps,
        core_ids=list(range(N_CORES)),
        tmpdir=os.environ.get("KERNEL_TMPDIR"),
    )
    LAST_RESULTS = res
    out = np.concatenate(
        [res.results[i]["out"].reshape(BPC, C) for i in range(N_CORES)], axis=0
    )
    return out.astype(np.float32)


# revision 20
# speedup vs baseline: 1.1918x; 1.1351x over previous
"""HardAttention kernel for Trainium2 (8 NeuronCores, Bass/Tile).

reference:
    scores = einsum("btd,bcsd->btcs", xs, ys)   # (B,Tx,C,Ty)
    out    = scores.max(-1).sum(1)              # (B,C)

Shapes: B=16, Tx=128, C=64, Ty=128, d=768.

Strategy:
  - Data-parallel over B: core i handles batches [2i, 2i+2).
  - Host pre-casts both operands to fp8 e4m3 (measured end-to-end rel
    err ~4e-3 vs the 2e-2 gate) and lays them out dk-major (partition
    dim first, contiguous DMA lines):
        xsT[dk, b, k, t]    = xs[b, t, 128k+dk]     (128, B, KC, Tx)
        ysT[b, dk, k, c, s] = ys[b, c, s, 128k+dk]  (B, 128, KC, C, Ty)
    so every DMA is a plain HWDGE copy (no dtype cast in the DMA path —
    SWDGE casting DMAs emit per-element descriptors and run ~5 GB/s).
  - Per (b, quarter-of-16-candidates): one 1.5 MB HWDGE DMA (contiguous
    2 KB runs per partition), then 3 (k-pair) x 4 (groups of 4
    candidates -> N=512) DoubleRow fp8 matmuls (2 contraction chunks
    per instruction, 0.5 cyc/row) accumulating into 4 PSUM banks; DVE
    reduce_max over Ty per candidate into an SBUF tile M[t, c]; finally
    a ones-vector matmul contracts the partition axis (sum over t) ->
    out[b, c].
"""

import os

import numpy as np

B, TX, C, TY, D = 16, 128, 64, 128, 768
N_CORES = 8
BPC = B // N_CORES          # batches per core = 2
KC = D // 128               # contraction chunks = 6
QC = 16                     # candidates per slab (DMA granule)
NQ = C // QC                # slabs per batch = 4
G = 4                       # candidates per matmul (N = G*TY = 512)

_CACHE = {}
LAST_RESULTS = None


def _build():
    import concourse.bass as bass
    import concourse.mybir as mybir
    import concourse.tile as tile
    from concourse import bacc

    fp8 = mybir.dt.float8e4
    f32 = mybir.dt.float32

    nc = bacc.Bacc(
        "TRN2",
        target_bir_lowering=False,
        debug=False,
        num_devices=N_CORES,
    )

    xs_ap = nc.dram_tensor("xsT", (128, BPC, KC, TX), fp8, kind="ExternalInput").ap()
    ys_ap = nc.dram_tensor(
        "ysT", (BPC, NQ, 128, KC, QC, TY), fp8, kind="ExternalInput"
    ).ap()
    out_ap = nc.dram_tensor("out", (1, BPC * C), f32, kind="ExternalOutput").ap()

    with tile.TileContext(nc) as tc:
        with (
            tc.tile_pool(name="xt", bufs=1) as xpool,
            tc.tile_pool(name="yt", bufs=2 * NQ - 1) as ypool,
            tc.tile_pool(name="yl", bufs=2) as ylpool,
            tc.tile_pool(name="mt", bufs=1) as mpool,
            tc.tile_pool(name="ones", bufs=1) as opool,
            tc.tile_pool(name="osb", bufs=1) as obpool,
            tc.tile_pool(name="ps", bufs=6, space="PSUM") as pspool,
            tc.tile_pool(name="pso", bufs=1, space="PSUM") as psopool,
        ):
            # All of xsT for this core: contiguous 1.5 KB per partition.
            xt = xpool.tile([128, BPC, KC, TX], fp8)
            nc.scalar.dma_start(xt[:], xs_ap)

            ones = opool.tile([128, 1], f32)
            nc.any.memset(ones[:], 1.0)

            m_all = mpool.tile([128, BPC, C], f32)  # max_s scores, [t, b, c]

            def do_group(b, c0, yt_ap, g, psum):
                # 3 accumulating DoubleRow matmuls + max over Ty for 4 cands
                for j in range(KC // 2):
                    nc.tensor.matmul(
                        psum[:],
                        lhsT=xt[:, b, 2 * j : 2 * j + 2, :],
                        rhs=yt_ap[:, 2 * j : 2 * j + 2, g * G : (g + 1) * G, :],
                        start=(j == 0),
                        stop=(j == KC // 2 - 1),
                        perf_mode=mybir.MatmulPerfMode.DoubleRow,
                    )

            for b in range(BPC):
                for q in range(NQ):
                    last = b == BPC - 1 and q == NQ - 1
                    if not last:
                        # slab: 12 KB fully contiguous per partition
                        yt = ypool.tile([128, KC, QC, TY], fp8)
                        dma_eng = nc.sync if (b * NQ + q) % 2 == 0 else nc.scalar
                        dma_eng.dma_start(yt[:], ys_ap[b, q])
                        halves = [(yt, 0), (yt, 1)]
                    else:
                        # final slab: two 6 KB half-DMAs so the tail's
                        # matmuls overlap the stream's last bytes
                        halves = []
                        for hh in range(2):
                            ytl = ylpool.tile(
                                [128, KC, QC // 2, TY], fp8, name=f"yl{hh}"
                            )
                            dma_eng = nc.sync if hh == 0 else nc.scalar
                            dma_eng.dma_start(
                                ytl[:],
                                ys_ap[b, q, :, :, hh * (QC // 2) : (hh + 1) * (QC // 2)],
                            )
                            halves.append((ytl, None))
                    for hh, (ysrc, hsel) in enumerate(halves):
                        psums = [
                            pspool.tile(
                                [128, G, TY], f32, name=f"ps_{b}_{q}_{hh}_{g}", tag="ps"
                            )
                            for g in range(2)
                        ]
                        for j in range(KC // 2):
                            for g in range(2):
                                gg = 2 * hsel + g if hsel is not None else g
                                src_g = gg if hsel is not None else g
                                nc.tensor.matmul(
                                    psums[g][:],
                                    lhsT=xt[:, b, 2 * j : 2 * j + 2, :],
                                    rhs=ysrc[:, 2 * j : 2 * j + 2, src_g * G : (src_g + 1) * G, :],
                                    start=(j == 0),
                                    stop=(j == KC // 2 - 1),
                                    perf_mode=mybir.MatmulPerfMode.DoubleRow,
                                )
                        for g in range(2):
                            cg = q * QC + (2 * hh + g) * G
                            nc.vector.reduce_max(
                                m_all[:, b, cg : cg + G],
                                psums[g][:],
                                axis=mybir.AxisListType.X,
                            )
            # sum over t (partition axis) via ones-vector matmul, both b at once
            out_ps = psopool.tile([1, BPC * C], f32, tag="out_ps")
            nc.tensor.matmul(
                out_ps[:], lhsT=ones[:], rhs=m_all[:], start=True, stop=True
            )
            osb = obpool.tile([1, BPC * C], f32, tag="osb")
            nc.vector.tensor_copy(osb[:], out_ps[:])
            nc.sync.dma_start(out_ap, osb[:])

    nc.compile()
    return nc


def _get_nc():
    if "nc" not in _CACHE:
        _CACHE["nc"] = _build()
    return _CACHE["nc"]


def _prep(xs: np.ndarray, ys: np.ndarray):
    """Host-side layout: fp8 cast + dk-major transpose (XLA on CPU)."""
    import jax
    import jax.numpy as jnp
    import ml_dtypes

    fp8 = ml_dtypes.float8_e4m3

    def _f(xs, ys):
        # xsT[dk, b, k, t] = xs[b, t, 128k+dk]
        xsT = jnp.transpose(
            jnp.reshape(xs.astype(fp8), (B, TX, KC, 128)), (3, 0, 2, 1)
        )
        # ysT[b, q, dk, k, cq, s] = ys[b, q*QC+cq, s, 128k+dk]
        ysT = jnp.transpose(
            jnp.reshape(ys.astype(fp8), (B, NQ, QC, TY, KC, 128)), (0, 1, 5, 4, 2, 3)
        )
        return xsT, ysT

    cpu = jax.devices("cpu")[0]
    with jax.default_device(cpu):
        xs_c = jax.device_put(np.ascontiguousarray(xs, dtype=np.float32), cpu)
        ys_c = jax.device_put(np.ascontiguousarray(ys, dtype=np.float32), cpu)
        xsT, ysT = jax.jit(_f)(xs_c, ys_c)
        return np.asarray(xsT), np.asarray(ysT)


def kernel(xs: np.ndarray, ys: np.ndarray) -> np.ndarray:
    global LAST_RESULTS
    from concourse.bass_utils import run_bass_kernel_spmd

    nc = _get_nc()
    xsT, ysT = _prep(xs, ys)
    in_maps = [
        {
            "xsT": np.ascontiguousarray(xsT[:, i * BPC : (i + 1) * BPC]),
            "ysT": ysT[i * BPC : (i + 1) * BPC],
        }
        for i in range(N_CORES)
    ]
    res = run_bass_kernel_spmd(
        nc,
        in_maps,
        core_ids=list(range(N_CORES)),
        tmpdir=os.environ.get("KERNEL_TMPDIR"),
    )
    LAST_RESULTS = res
    out = np.concatenate(
        [res.results[i]["out"].reshape(BPC, C) for i in range(N_CORES)], axis=0
    )
    return out.astype(np.float32)
